# revision 15
# baseline (speedup 1.0000x reference)
"""Raw-Bacc CenterLoss kernel, v2.

The masked distmat sum reduces to: loss = mean_b ||x_b - c_{label_b}||^2
(clip only affects the 9999 zero entries per row -> host-side constant).

Per core (128 batch rows), the device computes two [128,1] partials:
  col0: s1_p = sum_f x[p,f]^2 - 2*sum_f x[p,f]*c[p,f]   (DVE, 2 ttr passes)
  col1: s2_p = sum_f c[p,f]^2                           (ACT square+accum)
where c = centers[labels] via SWDGE indirect gather.

Timeline per core:
  SP (pre-barrier): labels [128,1] i32 DMA -> SBUF     (hoisted before the
      all-engine barrier so its ~2us completion latency overlaps startup)
  ACT: x [128,512] DMA; dummy activation to force the Square table load
      off the critical path
  DVE (hidden under gather): xx = rowsum(x*x)
  Pool: indirect gather centers[labels] -> c
  DVE: ttr  s1 = xx + rowsum((c*x)*-2)   ||  ACT: s2 = rowsum(square(c))
  SP: DMA [128,2] partials out
Host: clip per-row dist, sum 1024 partials, /B, + clip compensation.
"""

import os

import numpy as np

_BATCH = 1024
_FEAT = 512
_NCLASSES = 10000
_NCORES = 8
_ROWS = _BATCH // _NCORES  # 128
_P = 128

_state = {}

# knobs (A/B testable via env; defaults are the shipping config)
_PREBARRIER = os.environ.get("K_PREBARRIER", "1") == "1"
_ACT_WARMUP = os.environ.get("K_ACT_WARMUP", "1") == "1"
_USE_ACT = os.environ.get("K_USE_ACT", "1") == "1"


def _build_nc_raw():
    import concourse.bass as bass
    import concourse.mybir as mybir
    from concourse import bacc

    f32 = mybir.dt.float32
    i32 = mybir.dt.int32
    Alu = mybir.AluOpType
    Act = mybir.ActivationFunctionType

    nc = bacc.Bacc("TRN2", target_bir_lowering=False, debug=False)
    x_d = nc.dram_tensor("x", [_ROWS, _FEAT], f32, kind="ExternalInput").ap()
    labels_d = nc.dram_tensor("labels", [_ROWS, 1], i32, kind="ExternalInput").ap()
    centers_d = nc.dram_tensor(
        "centers", [_NCLASSES, _FEAT], f32, kind="ExternalInput"
    ).ap()
    out_d = nc.dram_tensor("out", [_P, 2], f32, kind="ExternalOutput").ap()

    from contextlib import ExitStack

    with ExitStack() as _es:
        ec = _es.enter_context
        labels_t = ec(nc.sbuf_tensor("labels_t", [_ROWS, 1], i32))
        x_t = ec(nc.sbuf_tensor("x_t", [_P, _FEAT], f32))
        c_t = ec(nc.sbuf_tensor("c_t", [_P, _FEAT], f32))
        junk_dve = ec(nc.sbuf_tensor("junk_dve", [_P, _FEAT], f32))
        junk_dve2 = ec(nc.sbuf_tensor("junk_dve2", [_P, _FEAT], f32))
        junk_act = ec(nc.sbuf_tensor("junk_act", [_P, _FEAT], f32))
        warm_t = ec(nc.sbuf_tensor("warm_t", [_P, 1], f32))
        xx_t = ec(nc.sbuf_tensor("xx_t", [_P, 1], f32))
        sxc_t = ec(nc.sbuf_tensor("sxc_t", [_P, 1], f32))
        part_t = ec(nc.sbuf_tensor("part_t", [_P, 2], f32))
        lab_sem = ec(nc.semaphore("lab_sem"))
        x_sem = ec(nc.semaphore("x_sem"))
        c_sem = ec(nc.semaphore("c_sem"))
        dve_sem = ec(nc.semaphore("dve_sem"))
        xx_sem = ec(nc.semaphore("xx_sem"))
        act_sem = ec(nc.semaphore("act_sem"))
        o_sem = ec(nc.semaphore("o_sem"))

        # labels on the SP HWDGE ring (hoisted pre-barrier below); x on the
        # ACT ring, gated on the first labels sem-incs so its bulk SDMA
        # traffic starts only after the tiny labels spray has drained.
        lab_dma = nc.sync.dma_start(labels_t.ap(), labels_d)
        lab_dma.then_inc(lab_sem, 16)
        nc.scalar.wait_ge(lab_sem, 4)
        nc.scalar.dma_start(x_t.ap(), x_d).then_inc(x_sem, 16)
        if _USE_ACT and _ACT_WARMUP:
            # tiny activation with no data deps: forces the Square table
            # load (~1.3us) to happen during the gather window. Reads the
            # framework's const-zero AP (initialized in the preamble).
            const0 = nc.const_aps.aps[(f32, 0.0)]
            nc.scalar.activation(out=warm_t.ap(), in_=const0, func=Act.Square)

        # gather c = centers[labels]
        nc.gpsimd.wait_ge(lab_sem, 16)
        nc.gpsimd.indirect_dma_start(
            out=c_t.ap(),
            out_offset=None,
            in_=centers_d,
            in_offset=bass.IndirectOffsetOnAxis(ap=labels_t.ap()[:, :1], axis=0),
        ).then_inc(c_sem, 16)

        # hidden under the gather: xx = rowsum(x*x)
        nc.vector.wait_ge(x_sem, 16)
        nc.vector.scalar_tensor_tensor(
            out=junk_dve.ap(),
            in0=x_t.ap(),
            scalar=1.0,
            in1=x_t.ap(),
            op0=Alu.mult,
            op1=Alu.mult,
            accum_out=xx_t.ap(),
        ).then_inc(xx_sem, 1)

        # post-gather: sxc = rowsum(-2*c*x), then s1 = sxc + xx  (DVE)
        nc.vector.wait_ge(c_sem, 16)
        nc.vector.scalar_tensor_tensor(
            out=junk_dve2.ap(),
            in0=c_t.ap(),
            scalar=-2.0,
            in1=x_t.ap(),
            op0=Alu.mult,
            op1=Alu.mult,
            accum_out=sxc_t.ap(),
        ).then_inc(dve_sem, 1)
        nc.vector.wait_ge(xx_sem, 1)
        nc.vector.wait_ge(dve_sem, 1)
        nc.vector.tensor_tensor(
            out=part_t.ap()[:, 0:1],
            in0=sxc_t.ap(),
            in1=xx_t.ap(),
            op=Alu.add,
        ).then_inc(dve_sem, 1)

        if _USE_ACT:
            # post-gather: s2 = rowsum(c^2)  (ACT, parallel with DVE)
            nc.scalar.wait_ge(c_sem, 16)
            nc.scalar.activation(
                out=junk_act.ap(),
                in_=c_t.ap(),
                func=Act.Square,
                accum_out=part_t.ap()[:, 1:2],
            ).then_inc(act_sem, 1)
        else:
            nc.vector.wait_ge(xx_sem, 1)
            nc.vector.scalar_tensor_tensor(
                out=junk_dve.ap(),
                in0=c_t.ap(),
                scalar=1.0,
                in1=c_t.ap(),
                op0=Alu.mult,
                op1=Alu.mult,
                accum_out=part_t.ap()[:, 1:2],
            ).then_inc(act_sem, 1)

        nc.sync.wait_ge(dve_sem, 2)
        nc.sync.wait_ge(act_sem, 1)
        nc.sync.dma_start(out_d, part_t.ap()).then_inc(o_sem, 16)

    if _PREBARRIER:
        # hoist the labels+x DMAs ahead of the all-engine start barrier:
        # insert them right after SP's barrier-arrival drain (which has
        # already bumped the barrier sem, so this does not delay other
        # engines) and before SP's barrier release wait.
        entry = nc.main_func.blocks[0]
        insts = entry.instructions
        sp = mybir.EngineType.SP
        sp_drain_idx = None
        for i, ins in enumerate(insts):
            if isinstance(ins, mybir.InstDrain) and ins.engine == sp:
                sp_drain_idx = i
                break
        if sp_drain_idx is not None:
            mv = lab_dma.ins
            if mv in insts and insts.index(mv) > sp_drain_idx:
                insts.remove(mv)
                insts.insert(sp_drain_idx + 1, mv)

    nc.compile()
    return nc


def _get_nc():
    if "nc" not in _state:
        _state["nc"] = _build_nc_raw()
    return _state["nc"]


def _postprocess(partials):
    """partials: list of [128,2] f32 arrays, one per core."""
    total = 0.0
    for p in partials:
        d = p[:, 0].astype(np.float64) + p[:, 1].astype(np.float64)
        d = np.clip(d, 1e-12, 1e12)
        total += float(d.sum())
    loss = total / _BATCH + (_NCLASSES - 1) * 1e-12
    return np.float32(loss)


def _run(x, labels, centers, trace=False):
    from concourse.bass_utils import run_bass_kernel_spmd

    nc = _get_nc()

    x = np.ascontiguousarray(np.asarray(x, dtype=np.float32)).reshape(
        _NCORES, _ROWS, _FEAT
    )
    lab = (
        np.ascontiguousarray(np.asarray(labels))
        .astype(np.int32)
        .reshape(_NCORES, _ROWS, 1)
    )
    cen = np.ascontiguousarray(np.asarray(centers, dtype=np.float32))
    in_maps = [{"x": x[i], "labels": lab[i], "centers": cen} for i in range(_NCORES)]
    res = run_bass_kernel_spmd(nc, in_maps, core_ids=list(range(_NCORES)), trace=trace)
    loss = _postprocess([r["out"] for r in res.results])
    return loss, res


def kernel(x, labels, centers):
    loss, _ = _run(x, labels, centers, trace=False)
    return loss
